# revision 22
# baseline (speedup 1.0000x reference)
"""BitLinear forward on 8 TRN2 NeuronCores — data-parallel over tokens.

Math (the activation-quant scale gamma cancels algebraically, as in the
original formulation; verified 6.6e-11 in f64):
    gamma_w = mean|W| + eps;  bw = clip(round(W/gamma_w), -1, 1)
    y = (LayerNorm(x) @ bw.T) * beta,   beta = max_d sum_o |W^T[d,o]|
No cross-core collective; each core handles 2048 tokens independently.

Structure (per core), tuned from perfetto/ntff traces:
  1. W streams in f32 on the sync DMA queue (gamma_w gates everything
     downstream); WRES tiles stay resident in SBUF, the rest re-DMA later.
     |W| row sums alternate scalar (Act.Abs + accum_out) and vector
     (tensor_reduce with apply_absolute_value) so the chain keeps pace
     with the ~2.5us/tile DMA arrivals.
  2. gamma_w/beta via two gpsimd partition reduces.
  3. Ternarize (scalar Sign + vector is_le*-2 + vector add), the ones-
     stationary colsum matmul, and the FIRST main-matmul group interleave
     per k-tile, so the PE starts the moment bw[0] exists and the W
     re-DMA tail hides behind it.
  4. x arrives as bf16 from the host in two layouts: token-blocked
     [m][d][k*tau] (one contiguous 4KiB/partition DMA per token group)
     for the matmul stationary, and natural [tok,d] rows for LN stats,
     which become free-dim reduces — no stats matmuls, no PSUM pressure,
     stats land directly columnized (no DRAM gather).  Chunk 0 computes
     entirely on scalar (idle after its Signs) so rbb/q are ready before
     the first epilogue while vector is still ternarizing.
  5. The rank-1 -mu*colsum LN correction is folded into a fused epilogue
     (scalar psum*rstd*beta/2, then one vector scalar_tensor_tensor
     y = cs2*q + ysb).  Stored ternary weights are s = 2*bw - 1 in
     {1,-1,-3}: any uniform offset c cancels exactly through the
     mu*colsum correction, so no bf16-exactness hacks are needed.
  6. The main matmul (m-groups of 128 tokens x 2048 outs, k inner) runs
     back-to-back at the warm-PE floor (~216ns per 512-col bf16 MM).

Measured on 8 cores: 328us traced (vs 483us traced for the previous
423.8us-harness baseline), rel err 1.5e-03 vs the f32 reference.

Hardware notes baked into the layout: the gpsimd/scalar DMA queues are
slow or stall-prone (software DGE / stream coupling) — all bulk DMA,
including the y writeback, stays on the sync hardware queue; vector-engine
abs ops (Alu.abs_max) fail neuronxcc codegen; gpsimd tensor_tensor is
~3.4x slower than vector and only worth using when vector saturates.
"""

import os
import sys

import numpy as np

for _p in ("/opt/trn_rl_repo", "/root/.axon_site/_ro/trn_rl_repo"):
    if os.path.isdir(_p) and _p not in sys.path:
        sys.path.append(_p)

from concourse import bacc, bass_isa, mybir, tile  # noqa: E402
from concourse.bass_utils import run_bass_kernel_spmd  # noqa: E402

import ml_dtypes  # noqa: E402

P = 128
D = 2048  # contraction (hidden) dim
O = 2048  # output dim
N_CORES = 8
N_TOK = 4 * 4096
TOK = N_TOK // N_CORES  # tokens per core
KT = D // P  # 16 contraction tiles
MT = TOK // P  # 16 token tiles per core
CH = 512  # psum free chunk (one bank of f32)
NCH = O // CH
WRES = 10  # W f32 tiles resident in SBUF (rest re-DMA'd for ternarize)
EPS = 1e-5
F32 = mybir.dt.float32
BF16 = mybir.dt.bfloat16
FP8 = mybir.dt.float8e4  # ternary weights {1,-1,-3} are exact in e4m3


def build_nc():
    nc = bacc.Bacc(None, target_bir_lowering=False, debug=False)
    # token-blocked x for the matmul: xm[m][p][k*128+tau] = x[t=128m+tau, d=128k+p]
    xm_d = nc.declare_dram_parameter("xm", [MT * P, KT * P], BF16, isOutput=False)
    # natural token-rows x for LN stats
    xr_d = nc.declare_dram_parameter("xr", [TOK, D], BF16, isOutput=False)
    fwt = nc.declare_dram_parameter("fwt", [D, O], F32, isOutput=False)
    y = nc.declare_dram_parameter("y", [TOK, O], F32, isOutput=True)

    Alu = mybir.AluOpType
    Act = mybir.ActivationFunctionType
    Ax = mybir.AxisListType

    with tile.TileContext(nc) as tc:
        with (
            tc.tile_pool(name="const", bufs=1) as const,
            tc.tile_pool(name="wres", bufs=WRES) as wres,
            tc.tile_pool(name="wstr", bufs=4) as wstr,
            tc.tile_pool(name="scr", bufs=2) as scr,
            tc.tile_pool(name="bneg", bufs=2) as bnegp,
            tc.tile_pool(name="bw", bufs=KT) as bwp,
            tc.tile_pool(name="xm", bufs=2) as xmp,
            tc.tile_pool(name="xrow", bufs=4) as xrp,
            tc.tile_pool(name="cs2", bufs=NCH) as cs2p,
            tc.tile_pool(name="ysb", bufs=3) as ypool,
            tc.tile_pool(name="yout", bufs=3) as youtp,
            tc.tile_pool(name="psum", bufs=8, space="PSUM") as psum,
        ):
            ones_b = const.tile([P, P], BF16)
            nc.vector.memset(ones_b, 1.0)
            eps_t = const.tile([P, 1], F32)
            nc.vector.memset(eps_t, EPS)
            scal = const.tile([P, 8], F32)  # scalar registry (columns)
            wsum = const.tile([P, KT], F32)  # per-d-row sum of |W| per tile
            # per-token stats, columnized: [P, MT]
            sx_c = const.tile([P, MT], F32)
            sq_c = const.tile([P, MT], F32)
            rbb_c = const.tile([P, MT], F32)  # rstd * beta/2
            q_c = const.tile([P, MT], F32)  # mu * rstd * beta/2

            # ---- phase A: W ingest + |W| row sums ----------------------
            wtiles = []
            for k in range(KT):
                pool = wres if k < WRES else wstr
                wk = pool.tile([P, O], F32, tag="wr" if k < WRES else "ws")
                nc.sync.dma_start(wk, fwt[P * k : P * (k + 1), :])
                # |W| row sums: alternate scalar Abs+accum and a single
                # vector abs-reduce so the chain keeps up with DMA arrivals
                if k % 2 == 0:
                    ab = scr.tile([P, O], BF16, tag="scr")
                    nc.scalar.activation(
                        ab, wk, Act.Abs, accum_out=wsum[:, k : k + 1]
                    )
                else:
                    nc.vector.tensor_reduce(
                        wsum[:, k : k + 1], wk, axis=Ax.X, op=Alu.add,
                        apply_absolute_value=True,
                    )
                wtiles.append(wk if k < WRES else None)

            # x DMAs for m0 + stats chunk 0 queue right behind W on sync
            xm0 = xmp.tile([P, KT * P], BF16, tag="xm")
            nc.sync.dma_start(xm0, xm_d[0:P, :])
            xr0 = []
            for s in range(4):
                xr = xrp.tile([P, D], BF16, tag="xr")
                nc.sync.dma_start(xr, xr_d[P * s : P * (s + 1), :])
                xr0.append(xr)
            # W re-DMA tail for ternarize (paced by wstr pool reuse)
            wb_tail = {}
            for k in range(WRES, KT):
                wk = wstr.tile([P, O], F32, tag="ws")
                nc.sync.dma_start(wk, fwt[P * k : P * (k + 1), :])
                wb_tail[k] = wk

            # ---- gamma_w / beta scalars --------------------------------
            row_tot = scal[:, 0:1]
            nc.vector.tensor_reduce(row_tot, wsum, axis=Ax.X, op=Alu.add)
            beta_pp = scal[:, 1:2]
            nc.vector.tensor_reduce(beta_pp, wsum, axis=Ax.X, op=Alu.max)
            tot_b = scal[:, 2:3]
            nc.gpsimd.partition_all_reduce(
                tot_b, row_tot, channels=P, reduce_op=bass_isa.ReduceOp.add
            )
            beta_b = scal[:, 3:4]
            nc.gpsimd.partition_all_reduce(
                beta_b, beta_pp, channels=P, reduce_op=bass_isa.ReduceOp.max
            )
            # thr = 0.5*gamma_w = 0.5*(tot/(D*O) + EPS)
            nthr = scal[:, 5:6]
            nc.scalar.activation(
                nthr, tot_b, Act.Copy, bias=-0.5 * EPS, scale=-0.5 / (D * O)
            )
            # epilogue is one fused op: y = (cs2 * q) + psum*rbb with
            # rbb = +beta/2 * rstd and q = -beta/2 * mu * rstd
            beta_hn = scal[:, 6:7]  # -beta/2
            nc.scalar.activation(beta_hn, beta_b, Act.Copy, bias=0.0, scale=-0.5)
            beta_hp = scal[:, 7:8]  # +beta/2
            nc.scalar.activation(beta_hp, beta_b, Act.Copy, bias=0.0, scale=0.5)

            # ---- stats chunk helper (vector/scalar, no PE) -------------
            def stats_chunk(c, xr_tiles):
                for s in range(4):
                    m = 4 * c + s
                    # chunk 0 computes entirely on scalar (idle after its
                    # Signs) so the busy vector stream isn't delayed and
                    # rbb/q are ready before the first epilogue
                    if c == 0:
                        sx = scr.tile([P, D], BF16, tag="scr")
                        nc.scalar.activation(
                            sx, xr_tiles[s], Act.Copy,
                            accum_out=sx_c[:, m : m + 1],
                        )
                    else:
                        nc.vector.tensor_reduce(
                            sx_c[:, m : m + 1], xr_tiles[s], axis=Ax.X,
                            op=Alu.add,
                        )
                    sq = scr.tile([P, D], BF16, tag="scr")
                    nc.scalar.activation(
                        sq, xr_tiles[s], Act.Square,
                        accum_out=sq_c[:, m : m + 1],
                    )
                sl = slice(4 * c, 4 * c + 4)
                # finalize on [P,4] slices (tiny)
                mu_t = scr_small[c][:, 0:4]
                nc.scalar.activation(
                    mu_t, sx_c[:, sl], Act.Copy, bias=0.0, scale=1.0 / D
                )
                ex2 = scr_small[c][:, 4:8]
                nc.scalar.activation(
                    ex2, sq_c[:, sl], Act.Copy, bias=0.0, scale=1.0 / D
                )
                musq = scr_small[c][:, 8:12]
                nc.scalar.activation(musq, mu_t, Act.Square)
                var = scr_small[c][:, 12:16]
                nc.vector.tensor_tensor(out=var, in0=ex2, in1=musq, op=Alu.subtract)
                nc.scalar.activation(var, var, Act.Sqrt, bias=eps_t)
                rstd = scr_small[c][:, 16:20]
                nc.vector.reciprocal(rstd, var)
                nc.vector.tensor_scalar(
                    out=rbb_c[:, sl], in0=rstd, scalar1=beta_hp, scalar2=None,
                    op0=Alu.mult,
                )
                qq = scr_small[c][:, 20:24]
                nc.vector.tensor_tensor(out=qq, in0=mu_t, in1=rstd, op=Alu.mult)
                nc.vector.tensor_scalar(
                    out=q_c[:, sl], in0=qq, scalar1=beta_hn, scalar2=None,
                    op0=Alu.mult,
                )

            scr_small = [
                const.tile([P, 24], F32, tag=f"ss{c}", name=f"ss{c}")
                for c in range(4)
            ]

            # ---- interleaved: ternarize k + colsum MM + main m0 MMs ----
            ps_cs = [
                psum.tile([P, CH], F32, tag="ps", name=f"cs{c}") for c in range(NCH)
            ]
            py0 = [
                psum.tile([P, CH], F32, tag="ps", name=f"py0_{c}") for c in range(NCH)
            ]
            bwts = []
            for k in range(KT):
                wk = wtiles[k] if k < WRES else wb_tail[k]
                bw = bwp.tile([P, O], FP8, tag="bw")
                # sgn = Sign(W - thr) in {-1,+1}
                nc.scalar.activation(bw, wk, Act.Sign, bias=nthr)
                bneg = bnegp.tile([P, O], FP8, tag="bneg")
                nc.vector.tensor_scalar(
                    out=bneg, in0=wk, scalar1=nthr, scalar2=-2.0,
                    op0=Alu.is_le, op1=Alu.mult,
                )
                # stored s = sgn + bneg in {1,-1,-3} = 2*bw - 1; the uniform
                # -1 offset cancels through the mu*colsum epilogue correction
                nc.vector.tensor_tensor(out=bw, in0=bw, in1=bneg, op=Alu.add)
                bwts.append(bw)
                first, last = k == 0, k == KT - 1
                for c in range(NCH):
                    sl = slice(CH * c, CH * (c + 1))
                    nc.tensor.matmul(
                        ps_cs[c], ones_b, bw[:, sl], start=first, stop=last
                    )
                for c in range(NCH):
                    sl = slice(CH * c, CH * (c + 1))
                    nc.tensor.matmul(
                        py0[c], xm0[:, P * k : P * (k + 1)], bw[:, sl],
                        start=first, stop=last,
                    )

            # colsum -> SBUF f32 (broadcast over partitions already)
            cs2 = []
            for c in range(NCH):
                ct = cs2p.tile([P, CH], F32, tag="cs2")
                nc.vector.tensor_copy(out=ct, in_=ps_cs[c])
                cs2.append(ct)

            stats_chunk(0, xr0)

            # ---- epilogue helper ---------------------------------------
            def epilogue(m, pys):
                for c in range(NCH):
                    ysb = ypool.tile([P, CH], F32, tag="y")
                    nc.scalar.mul(ysb, pys[c], rbb_c[:, m : m + 1])
                    yo = youtp.tile([P, CH], F32, tag="yo")
                    nc.vector.scalar_tensor_tensor(
                        out=yo, in0=cs2[c], scalar=q_c[:, m : m + 1], in1=ysb,
                        op0=Alu.mult, op1=Alu.add,
                    )
                    # sync queue is hardware-DGE and idle once inputs are
                    # in; gpsimd's software queue drags the writeback tail
                    nc.sync.dma_start(
                        y[P * m : P * (m + 1), CH * c : CH * (c + 1)], yo
                    )

            epilogue(0, py0)

            # ---- main loop m = 1..15 -----------------------------------
            xr_pending = {}
            for m in range(1, MT):
                xmt = xmp.tile([P, KT * P], BF16, tag="xm")
                nc.sync.dma_start(xmt, xm_d[P * m : P * (m + 1), :])
                # stats chunks 1..3: DMA xrows early, compute when queued
                if m in (1, 5, 9):
                    cc = (m + 3) // 4
                    tiles = []
                    for s in range(4):
                        xr = xrp.tile([P, D], BF16, tag="xr")
                        nc.sync.dma_start(
                            xr, xr_d[P * (4 * cc + s) : P * (4 * cc + s + 1), :]
                        )
                        tiles.append(xr)
                    xr_pending[cc] = tiles
                pys = [
                    psum.tile([P, CH], F32, tag="ps", name=f"py{m}_{c2}")
                    for c2 in range(NCH)
                ]
                for k in range(KT):
                    first, last = k == 0, k == KT - 1
                    lhs = xmt[:, P * k : P * (k + 1)]
                    for c2 in range(NCH):
                        nc.tensor.matmul(
                            pys[c2], lhs, bwts[k][:, CH * c2 : CH * (c2 + 1)],
                            start=first, stop=last,
                        )
                if m in (2, 6, 10):
                    cc = (m + 2) // 4
                    stats_chunk(cc, xr_pending.pop(cc))
                epilogue(m, pys)

    nc.compile()
    return nc


_NC_CACHE = None


def _get_nc():
    global _NC_CACHE
    if _NC_CACHE is None:
        _NC_CACHE = build_nc()
    return _NC_CACHE


def _prep_in_maps(x, fweight):
    bf16 = ml_dtypes.bfloat16
    x2 = np.ascontiguousarray(x, dtype=np.float32).reshape(N_TOK, D)
    fwt = np.ascontiguousarray(np.asarray(fweight, dtype=np.float32).T)
    in_maps = []
    for c in range(N_CORES):
        xc = x2[c * TOK : (c + 1) * TOK, :]
        xr = np.ascontiguousarray(xc).astype(bf16)
        xmb = np.ascontiguousarray(
            xc.reshape(MT, P, KT, P).transpose(0, 3, 2, 1)
        ).astype(bf16).reshape(MT * P, KT * P)
        in_maps.append({"xm": xmb, "xr": xr, "fwt": fwt})
    return in_maps


def run_spmd(x, fweight, **kw):
    nc = _get_nc()
    in_maps = _prep_in_maps(x, fweight)
    return run_bass_kernel_spmd(nc, in_maps, core_ids=list(range(N_CORES)), **kw)


def kernel(x, fweight):
    res = run_spmd(x, fweight)
    y = np.concatenate([res.results[c]["y"] for c in range(N_CORES)], axis=0)
    return y.reshape(4, 4096, O)


if __name__ == "__main__":
    xx = np.random.randn(4, 4096, D).astype(np.float32)
    ww = np.random.uniform(-1 / np.sqrt(D), 1 / np.sqrt(D), (O, D)).astype(np.float32)
    out = kernel(xx, ww)
    print("out", out.shape, out.dtype, float(np.abs(out).mean()))
